# revision 1
# baseline (speedup 1.0000x reference)
"""Trainium2 Bass kernel for the topopt compliance-loss problem.

Strategy (structured fast path):
  The reference's edofMat is the standard Q4 grid connectivity, so the
  gather U[:, edofMat] is a 2x2 node stencil over the displacement field
  viewed as a [513, 513, 2] node image.  Per element (x, y):
      ce[y, x] = u^T K u,   u = 8 DOFs of the 4 corner nodes
  With K = sym(KE) = V diag(lam) V^T (host eigh), ce = sum_r sign(lam_r)*G_r^2
  where G_r = a_r . u is a *linear* stencil -> computed on the TensorEngine
  as banded matmuls over the transposed node image (partitions = node rows,
  so the (dy, c) DOF offsets become partition offsets inside the matmul
  contraction window; the dx offset is a free-dim shift of the rhs).

  Per core: 2 batches (pure data parallel over B=16 on 8 cores).
  Dtypes: all matmuls run in float32r (4x cheaper than fp32 on the PE).
  The naive f32r W-rounding bias (~6e-5) is eliminated by quantizing the
  SOS vectors onto the measured f32r storage grid (round-to-nearest,
  12-bit mantissa) with error-compensating coordinate descent on the host,
  so the device-side convert is exact; U-data rounding is unbiased noise
  that averages out (~5e-6 end-to-end, verified on hardware).
  Device emits per-partition partial sums; host does the final O(B) scalars.

Fallback: any input not matching the structured grid (edofMat/penal/shape)
is computed on host in float64 numpy (same semantics as the reference).
"""

import sys

for _p in ('/opt/trn_rl_repo', '/opt/trn_rl_repo/concourse'):
    if _p not in sys.path:
        sys.path.insert(0, _p)

import numpy as np

B, NX, NY, NN = 16, 512, 512, 513
NDOF = 2 * NN * NN
NELE = NX * NY
N_CORES = 8
BPC = B // N_CORES  # batches per core
EMIN, EMAX = 1e-9, 1.0
DE = EMAX - EMIN

# edofMat column -> (dx, dy, c) node-stencil offsets (derived from the Q4
# connectivity: cols [2n1+2, 2n1+3, 2n2+2, 2n2+3, 2n2, 2n2+1, 2n1, 2n1+1])
COL_AX = (0, 0, 1, 1, 1, 1, 0, 0)
COL_AY = (1, 1, 1, 1, 0, 0, 0, 0)
COL_C = (0, 1, 0, 1, 0, 1, 0, 1)

N_PT = 11          # transposed-node-image tiles, partition stride 96
PT_W = 520         # free width (513 used)
N_YT = 4           # y-tiles of 128 per batch
# output partials column layout (per core, [128, 32]):
#   cols  i*8 + k        : compliance accumulation chain, batch i (8 links)
#   cols 16 + i*4 + yt   : rho partial sums
#   cols 24 + i*4 + yt   : vol partial sums
OUT_COLS = 32


def _build_edof():
    elx = np.repeat(np.arange(NX), NY)
    ely = np.tile(np.arange(NY), NX)
    n1 = (NY + 1) * elx + ely
    n2 = (NY + 1) * (elx + 1) + ely
    return np.stack([2 * n1 + 2, 2 * n1 + 3, 2 * n2 + 2, 2 * n2 + 3,
                     2 * n2, 2 * n2 + 1, 2 * n1, 2 * n1 + 1], axis=1)


def _build_consts(KE):
    """W0/W1 banded stencil matrices and the signed sum-selector S."""
    K = (KE.astype(np.float64) + KE.astype(np.float64).T) / 2
    lam, V = np.linalg.eigh(K)
    a = V * np.sqrt(np.abs(lam))[None, :]      # a[:, r]
    s = np.sign(lam)

    # Quantize the SOS vectors onto the f32r storage grid (round-to-nearest,
    # 12-bit mantissa — measured on device) with error-compensating
    # coordinate descent so sum_r s_r a_q a_q^T stays close to K.  The
    # device-side f32 -> f32r convert is then exact, letting the G-matmuls
    # run in f32r (4x cheaper than fp32) without the rounding bias.
    def _q12(v):
        m, e = np.frexp(np.float64(v))
        return np.round(m * 4096.0) / 4096.0 * 2.0 ** e

    def _ulp12(v):
        _, e = np.frexp(np.float64(v) if v != 0 else 1e-12)
        return 2.0 ** e / 4096.0

    aq = np.vectorize(_q12)(a)
    best = np.linalg.norm(K - (aq * s[None, :]) @ aq.T)
    for _ in range(40):
        improved = False
        for i in range(8):
            for r in range(8):
                v0 = aq[i, r]
                u = _ulp12(v0 if v0 != 0 else a[i, r])
                for k in (-3, -2, -1, 1, 2, 3):
                    aq[i, r] = v0 + k * u
                    n = np.linalg.norm(K - (aq * s[None, :]) @ aq.T)
                    if n < best - 1e-18:
                        best = n
                        v0 = aq[i, r]
                        improved = True
                aq[i, r] = v0
        if not improved:
            break
    a = aq
    W = np.zeros((2, 34, 128), np.float32)
    for r in range(8):
        for y16 in range(16):
            m = r * 16 + y16
            for i in range(8):
                W[COL_AX[i], 2 * y16 + 2 * COL_AY[i] + COL_C[i], m] += a[i, r]
    # 8 selector variants: S[:, j*128:(j+1)*128] maps SQ-pack rows (r, y16)
    # to ce rows 16j + y16 (zeros elsewhere; ce accumulates over j in PSUM)
    S = np.zeros((128, 8 * 128), np.float32)
    for j in range(8):
        for r in range(8):
            for y16 in range(16):
                S[r * 16 + y16, j * 128 + 16 * j + y16] = s[r]
    return W, S


def _numpy_fallback(rho, U, vol_field, solid_comp, KE, edofMat, penal, lambda_vol):
    rho64 = rho.astype(np.float64)
    U64 = U.astype(np.float64)
    Ue = U64[:, edofMat]                      # [B, nele, 8]
    ce = np.einsum('bei,ij,bej->be', Ue, KE.astype(np.float64), Ue)
    nb, nely, nelx = rho.shape
    ce = ce.reshape(nb, nelx, nely).transpose(0, 2, 1)
    compliance = ((EMIN + rho64 ** penal * (EMAX - EMIN)) * ce).sum(axis=(1, 2))
    n_ele = nelx * nely
    volfrac = vol_field.astype(np.float64).sum(axis=(1, 2)) / n_ele
    viol = np.abs(rho64.sum(axis=(1, 2)) / n_ele - volfrac)
    loss = compliance / solid_comp.astype(np.float64) + lambda_vol * viol
    return (loss.astype(np.float32), compliance.astype(np.float32),
            viol.astype(np.float32))


_NC_CACHE = {}


def _build_nc():
    if 'nc' in _NC_CACHE:
        return _NC_CACHE['nc']
    import os
    SKIP = set(os.environ.get('BASSK_SKIP', '').split(','))
    from contextlib import ExitStack
    from concourse import bass, mybir, tile

    f32 = mybir.dt.float32
    f32r = mybir.dt.float32r
    Copy = mybir.ActivationFunctionType.Copy
    nc = bass.Bass("TRN2", target_bir_lowering=False, debug=False)
    p_u = nc.declare_dram_parameter("u", [BPC, NN, 2 * NN], f32, isOutput=False)
    p_rho = nc.declare_dram_parameter("rho", [BPC, NY, NX], f32, isOutput=False)
    p_vol = nc.declare_dram_parameter("vol", [BPC, NY, NX], f32, isOutput=False)
    p_w = nc.declare_dram_parameter("wmat", [128, 768], f32, isOutput=False)
    p_s = nc.declare_dram_parameter("smat", [128, 1024], f32, isOutput=False)
    p_id = nc.declare_dram_parameter("ident", [128, 128], f32, isOutput=False)
    p_out = nc.declare_dram_parameter("partials", [128, OUT_COLS], f32,
                                      isOutput=True)

    with tile.TileContext(nc) as tc, ExitStack() as ctx:
        consts = ctx.enter_context(tc.tile_pool(name="consts", bufs=1))
        ni_p = ctx.enter_context(tc.tile_pool(name="ni", bufs=9))
        pt_p = ctx.enter_context(tc.tile_pool(name="pt", bufs=2 * N_PT + 2))
        sq_p = ctx.enter_context(tc.tile_pool(name="sq", bufs=6))
        fld_p = ctx.enter_context(tc.tile_pool(name="fld", bufs=4))
        ps_tr = ctx.enter_context(tc.tile_pool(name="pstr", bufs=2, space="PSUM"))
        ps_g = ctx.enter_context(tc.tile_pool(name="psg", bufs=3, space="PSUM"))
        ps_ce = ctx.enter_context(tc.tile_pool(name="psce", bufs=2, space="PSUM"))
        ps_tc = ctx.enter_context(tc.tile_pool(name="pstc", bufs=1, space="PSUM"))

        wmat_f = consts.tile([128, 768], f32)
        wmat = consts.tile([128, 768], f32r)
        smat_f = consts.tile([128, 1024], f32)
        smat = consts.tile([128, 1024], f32r)
        ident = consts.tile([128, 128], f32)
        out_t = consts.tile([128, OUT_COLS], f32)
        zero_c = consts.tile([128, 1], f32)
        junk = consts.tile([128, 512], f32)
        nc.sync.dma_start(out=wmat_f[:], in_=p_w[:])
        nc.vector.tensor_copy(out=wmat[:], in_=wmat_f[:])
        nc.sync.dma_start(out=smat_f[:], in_=p_s[:])
        nc.vector.tensor_copy(out=smat[:], in_=smat_f[:])
        nc.sync.dma_start(out=ident[:], in_=p_id[:])
        nc.vector.memset(zero_c[:], 0.0)

        all_pt = {}
        for bi in range(BPC):
            # ---- build PT tiles (transposed node image) ----
            ni_tiles = []
            for xc in range(4):
                t = ni_p.tile([128, 2 * NN], f32, tag="ni")
                nc.sync.dma_start(out=t[:], in_=p_u[bi, 128 * xc:128 * (xc + 1), :])
                ni_tiles.append(t)
            ni4 = ni_p.tile([1, 2 * NN], f32, tag="ni4")
            nc.sync.dma_start(out=ni4[:], in_=p_u[bi, NN - 1:NN, :])

            pt_tiles = []
            for t_i in range(0 if 'nopt' in SKIP else N_PT):
                w = min(128, 2 * NN - 96 * t_i)
                pt = pt_p.tile([128, PT_W], f32r, tag="pt")
                if w < 128 and 'memset' not in SKIP:
                    nc.gpsimd.memset(pt[:].bitcast(f32), 0.0)
                stage = ps_tr.tile([128, 512], f32, tag="pstr")
                for xc in range(4):
                    nc.tensor.transpose(
                        out=stage[:w, 128 * xc:128 * (xc + 1)],
                        in_=ni_tiles[xc][:, 96 * t_i:96 * t_i + w],
                        identity=ident[:],
                    )
                if t_i % 2 == 0:
                    nc.vector.tensor_copy(out=pt[:w, 0:512], in_=stage[:w, :])
                else:
                    nc.scalar.copy(out=pt[:w, 0:512], in_=stage[:w, :])
                # last node-column (x = 512) via a tiny transpose
                tinyps = ps_tc.tile([128, 1], f32, tag="pstc")
                nc.tensor.transpose(
                    out=tinyps[:w, 0:1],
                    in_=ni4[0:1, 96 * t_i:96 * t_i + w],
                    identity=ident[0:1, 0:1],
                )
                nc.vector.tensor_copy(out=pt[:w, 512:513], in_=tinyps[:w, 0:1])
                pt_tiles.append(pt)
            all_pt[bi] = pt_tiles

        for bi in range(BPC):
            pt_tiles = all_pt[bi]
            # ---- per y-tile: weights field, sums, stencil matmuls ----
            for yt in range(N_YT):
                ysl = slice(128 * yt, 128 * (yt + 1))
                if 'nofld' in SKIP:
                    rt = fld_p.tile([128, 512], f32, tag="rho")
                else:
                    rt = fld_p.tile([128, 512], f32, tag="rho")
                vt = fld_p.tile([128, 512], f32, tag="vol")
                nc.sync.dma_start(out=rt[:], in_=p_rho[bi, ysl, :])
                nc.sync.dma_start(out=vt[:], in_=p_vol[bi, ysl, :])
                r2 = fld_p.tile([128, 512], f32, tag="r2")
                r3 = fld_p.tile([128, 512], f32, tag="r3")
                wt = fld_p.tile([128, 512], f32, tag="wt")
                nc.scalar.square(out=r2[:], in_=rt[:])
                if 'gmul' in SKIP:
                    nc.vector.tensor_mul(r3[:], r2[:], rt[:])
                else:
                    nc.gpsimd.tensor_mul(r3[:], r2[:], rt[:])
                # w = EMIN + DE * rho^3
                nc.scalar.activation(wt[:], r3[:], Copy, bias=EMIN, scale=DE)
                # partial sums of rho and vol (over x) via ACT accumulators
                nc.scalar.activation(
                    junk[:], rt[:], Copy, bias=0.0, scale=1.0,
                    accum_out=out_t[:, 16 + bi * 4 + yt: 17 + bi * 4 + yt])
                nc.scalar.activation(
                    junk[:], vt[:], Copy, bias=0.0, scale=1.0,
                    accum_out=out_t[:, 24 + bi * 4 + yt: 25 + bi * 4 + yt])

                if 'nog' in SKIP:
                    continue
                ce = ps_ce.tile([128, 512], f32, tag="psce", name="ce")
                for j in range(8):
                    mi = 8 * yt + j
                    t_i = mi // 3
                    g = ps_g.tile([128, 512], f32, tag="psg")
                    v = mi % 3
                    nc.tensor.matmul(
                        out=g[:],
                        lhsT=wmat[:, v * 256: v * 256 + 128],
                        rhs=pt_tiles[t_i][:, 0:512],
                        start=True, stop=False)
                    nc.tensor.matmul(
                        out=g[:],
                        lhsT=wmat[:, v * 256 + 128: v * 256 + 256],
                        rhs=pt_tiles[t_i][:, 1:513],
                        start=False, stop=True)
                    if 'nosq' in SKIP:
                        continue
                    sq = sq_p.tile([128, 512], f32r, tag="sq")
                    nc.scalar.square(out=sq[:], in_=g[:])
                    # selector j places this pack's rows at 16j + y16;
                    # all 8 packs accumulate into one full-height ce tile
                    nc.tensor.matmul(
                        out=ce[:],
                        lhsT=smat[:, j * 128:(j + 1) * 128],
                        rhs=sq[:],
                        start=(j == 0), stop=(j == 7))
                # weighted reduce; each y-tile gets its own output column
                # (host sums the 4 columns per batch)
                scratch = fld_p.tile([128, 512], f32, tag="scr")
                nc.vector.scalar_tensor_tensor(
                    out=scratch[:],
                    in0=ce[:],
                    scalar=1.0,
                    in1=wt[:],
                    op0=mybir.AluOpType.mult,
                    op1=mybir.AluOpType.mult,
                    accum_out=out_t[:, bi * 4 + yt: bi * 4 + yt + 1])

        nc.sync.dma_start(out=p_out[:], in_=out_t[:])

    # walrus in this container rejects >1 sem-wait per instruction; split.
    _split_waits(nc)
    _NC_CACHE['nc'] = nc
    return nc


def _split_waits(nc):
    from concourse import mybir
    drainable = {"PE", "DVE", "Activation", "Pool", "SP"}
    n = 0
    for f in nc.m.functions:
        for bb in f.blocks:
            insts = list(bb.instructions)
            new_list = []
            changed = False
            for ins in insts:
                si = ins.sync_info
                waits = list(si.on_wait) if si is not None and si.on_wait else []
                eng = str(ins.engine).split(".")[-1]
                if len(waits) > 1 and eng in drainable:
                    changed = True
                    for w in waits[:-1]:
                        d = mybir.InstDrain(name=f"{ins.name}-ws{n}", ins=[], outs=[])
                        d.engine = ins.engine
                        d.sync_info = mybir.SyncInfo(on_wait=[w], on_update=[])
                        new_list.append(d)
                        n += 1
                    ins.sync_info = mybir.SyncInfo(
                        on_wait=[waits[-1]],
                        on_update=list(si.on_update) if si.on_update else [])
                new_list.append(ins)
            if changed:
                bb.instructions = new_list
    return n


def kernel(rho, U, vol_field, solid_comp, KE, edofMat, penal, lambda_vol):
    rho = np.asarray(rho, np.float32)
    U = np.asarray(U, np.float32)
    vol = np.asarray(vol_field, np.float32)
    sc = np.asarray(solid_comp, np.float32)
    KEn = np.asarray(KE, np.float32)
    ed = np.asarray(edofMat)
    pen = int(np.asarray(penal))
    lv = float(np.asarray(lambda_vol))

    structured = (
        rho.shape == (B, NY, NX) and U.shape == (B, NDOF)
        and vol.shape == (B, NY, NX) and ed.shape == (NELE, 8)
        and pen == 3
        and np.array_equal(ed.astype(np.int64), _build_edof())
    )
    if not structured:
        return _numpy_fallback(rho, U, vol, sc, KEn,
                               ed.astype(np.int64), pen, lv)

    from concourse.bass_utils import run_bass_kernel_spmd

    W, S = _build_consts(KEn)
    wmat = np.zeros((128, 768), np.float32)
    for v in range(3):
        for ax in range(2):
            wmat[32 * v:32 * v + 34, v * 256 + ax * 128: v * 256 + (ax + 1) * 128] = W[ax]
    ident = np.eye(128, dtype=np.float32)

    nc = _build_nc()
    in_maps = []
    for c in range(N_CORES):
        bsl = slice(BPC * c, BPC * (c + 1))
        in_maps.append({
            "u": np.ascontiguousarray(U[bsl].reshape(BPC, NN, 2 * NN)),
            "rho": np.ascontiguousarray(rho[bsl]),
            "vol": np.ascontiguousarray(vol[bsl]),
            "wmat": wmat,
            "smat": S,
            "ident": ident,
        })
    res = run_bass_kernel_spmd(nc, in_maps, list(range(N_CORES)))
    _NC_CACHE['last_result'] = res

    compliance = np.zeros(B, np.float64)
    rho_sum = np.zeros(B, np.float64)
    vol_sum = np.zeros(B, np.float64)
    for c in range(N_CORES):
        p = res.results[c]["partials"].astype(np.float64)
        for i in range(BPC):
            b = BPC * c + i
            compliance[b] = p[:, i * 4: i * 4 + 4].sum()
            rho_sum[b] = p[:, 16 + i * 4: 16 + i * 4 + 4].sum()
            vol_sum[b] = p[:, 24 + i * 4: 24 + i * 4 + 4].sum()
    volfrac = vol_sum / NELE
    viol = np.abs(rho_sum / NELE - volfrac)
    loss = compliance / sc.astype(np.float64) + lv * viol
    return (loss.astype(np.float32), compliance.astype(np.float32),
            viol.astype(np.float32))

